# revision 6
# baseline (speedup 1.0000x reference)
"""CRF loss kernel for Trainium2 (8 NeuronCores, data-parallel over batch).

Reference computation:
    score = einsum('blf,fk->blk', X, W);  forward CRF recursion over L;
    loss = mean_b(emit + trans - logZ).

Design (v2) — minimize NEFF input bytes and instruction count:
  - X is pre-transposed and quantized to fp8e4 on the host:
    XT[f, g*8192 + t*256 + b] = X[core*1024 + g*256 + b, t, f].
    One [128, 32768] fp8 DMA per core (4.2 MB vs 16.8 MB f32), no
    on-device transposes.
  - score matmul: lhsT = W block [128,32] bf16, rhs = XT 512-col chunks,
    4 label-row groups packed at partition offsets 0/32/64/96
    (tile_position).  8 PSUM rounds of [128,1024] f32.
  - expsc = exp(score - SHIFT) via one ACT per round (PSUM -> SBUF bf16).
  - emit = sum(score[gold]): per round, mask * score (DVE mult from PSUM)
    then tensor_reduce(add) into an f32 accumulator column.
    mask = is_equal(yrep, kidx) built on device from a tiny [4, 8192]
    bf16 y input (partition-broadcast DMA, stride-0 source).
  - CRF forward recursion in probability domain:
      p_t = (BD^T p_{t-1}) * expsc_t,  BD = block-diag(exp(T)) bf16,
    renormalized every RN=4 steps (z = ZS^T p group-sums, ln(z)
    accumulated via ACT Ln accum_out).
  - trans = sum T[y_t, y_{t+1}] is computed on the host (y and T only).
  - per-core output: [128, 16] f32 = 8 ln-accum cols + 8 emit-accum cols;
    host combines: loss = (sum emit + trans - sum(ln)/32 - BC*L*SHIFT)/B.
"""

import numpy as np

B, L, F, K = 8192, 32, 128, 26
N_CORES = 8
BC = B // N_CORES            # 1024 batch per core
GROUPS = 4                   # label-row groups packed on partitions
GB = BC // GROUPS            # 256 batch columns per group
NT = L * GB                  # 8192 columns in expsc/mask layout
SHIFT = 26.0
RN = 4                       # renormalize every RN recursion steps
PSC = 1024                   # PSUM round width (f32 cols, 2 banks)
NROUND = NT // PSC           # 8 score rounds
NLN = (L - 1) // RN + 1      # 7 renorms + 1 final ln column

_cache = {}


def _build_program():
    import concourse.bass as bass  # noqa: F401
    import concourse.bacc as bacc
    import concourse.tile as tile
    from concourse import mybir
    from contextlib import ExitStack

    f32 = mybir.dt.float32
    bf16 = mybir.dt.bfloat16
    fp8 = mybir.dt.float8e4
    AF = mybir.ActivationFunctionType
    ALU = mybir.AluOpType

    nc = bacc.Bacc("TRN2", target_bir_lowering=False)

    XTd = nc.dram_tensor("XT", [128, GROUPS * NT], fp8, kind="ExternalInput")
    YARRd = nc.dram_tensor("YARR", [GROUPS, NT], bf16, kind="ExternalInput")
    CONSTd = nc.dram_tensor("CONST", [128, 289], bf16, kind="ExternalInput")
    OUTd = nc.dram_tensor("out", [128, 16], f32, kind="ExternalOutput")

    with tile.TileContext(nc) as tc, ExitStack() as ctx:
        singles = ctx.enter_context(tc.tile_pool(name="singles", bufs=1))

        xt = singles.tile([128, GROUPS * NT], fp8)
        nc.sync.dma_start(out=xt, in_=XTd.ap())
        cst = singles.tile([128, 289], bf16)
        nc.sync.dma_start(out=cst, in_=CONSTd.ap())
        wblk = cst[:, 0:32]
        bd = cst[:, 32:160]
        zs = cst[:, 160:288]
        kidx = cst[:, 288:289]

        yrep = singles.tile([128, NT], bf16)
        for g in range(GROUPS):
            nc.sync.dma_start(
                out=yrep[32 * g:32 * (g + 1)],
                in_=YARRd.ap()[g:g + 1, :].to_broadcast([32, NT]),
            )
        mask = singles.tile([128, NT], bf16)
        nc.vector.tensor_tensor(
            mask, yrep, kidx.to_broadcast([128, NT]), ALU.is_equal
        )

        expsc = singles.tile([128, NT], bf16)
        acc = singles.tile([128, 16], f32)
        nshift = singles.tile([128, 1], f32)
        nc.vector.memset(nshift, -SHIFT)

        # ---------------- phase 1: score / exp / emit ----------------
        with tc.tile_pool(name="scp", bufs=2, space="PSUM") as scp, \
             tc.tile_pool(name="emp", bufs=2) as emp:
            for r in range(NROUND):
                ps = scp.tile([128, PSC], f32)
                for g in range(GROUPS):
                    for j in range(PSC // 512):
                        col = g * NT + r * PSC + j * 512
                        nc.tensor.matmul(
                            ps[32 * g:32 * g + 32, j * 512:(j + 1) * 512],
                            lhsT=wblk,
                            rhs=xt[:, col:col + 512],
                            start=True, stop=True,
                            tile_position=(0, 32 * g),
                        )
                nc.scalar.activation(
                    expsc[:, r * PSC:(r + 1) * PSC], ps, AF.Exp, bias=nshift
                )
                emsc = emp.tile([128, PSC], f32)
                nc.vector.tensor_tensor(
                    emsc, ps, mask[:, r * PSC:(r + 1) * PSC], ALU.mult
                )
                nc.vector.tensor_reduce(
                    acc[:, 8 + r:9 + r], emsc,
                    axis=mybir.AxisListType.X, op=ALU.add,
                )

        # ---------------- phase 2: CRF recursion ----------------
        with tc.tile_pool(name="pp", bufs=2) as pp, \
             tc.tile_pool(name="vp", bufs=2) as vp, \
             tc.tile_pool(name="rzp", bufs=2) as rzp, \
             tc.tile_pool(name="lnp", bufs=2) as lnp, \
             tc.tile_pool(name="up", bufs=2, space="PSUM") as up, \
             tc.tile_pool(name="zp", bufs=2, space="PSUM") as zp:
            p_prev = expsc[:, 0:GB]
            nidx = 0
            for t in range(1, L):
                u = up.tile([128, GB], f32)
                nc.tensor.matmul(u, lhsT=bd, rhs=p_prev, start=True, stop=True)
                e_sl = expsc[:, t * GB:(t + 1) * GB]
                if t % RN == 0:
                    v = vp.tile([128, GB], bf16)
                    nc.vector.tensor_mul(v, u, e_sl)
                    z = zp.tile([128, GB], f32)
                    nc.tensor.matmul(z, lhsT=zs, rhs=v, start=True, stop=True)
                    lnscr = lnp.tile([128, GB], bf16)
                    nc.scalar.activation(
                        lnscr, z, AF.Ln, accum_out=acc[:, nidx:nidx + 1]
                    )
                    nidx += 1
                    rz = rzp.tile([128, GB], f32)
                    nc.vector.reciprocal(rz, z)
                    pn = pp.tile([128, GB], bf16)
                    nc.vector.tensor_mul(pn, v, rz)
                else:
                    pn = pp.tile([128, GB], bf16)
                    nc.vector.tensor_mul(pn, u, e_sl)
                p_prev = pn
            zf = zp.tile([128, GB], f32)
            nc.tensor.matmul(zf, lhsT=zs, rhs=p_prev, start=True, stop=True)
            lnscr = lnp.tile([128, GB], bf16)
            nc.scalar.activation(
                lnscr, zf, AF.Ln, accum_out=acc[:, nidx:nidx + 1]
            )
            nidx += 1
            assert nidx == NLN

        nc.sync.dma_start(out=OUTd.ap(), in_=acc)

    nc.compile()
    return nc


def _get_program():
    if "nc" not in _cache:
        _cache["nc"] = _build_program()
    return _cache["nc"]


def _make_consts(W, T):
    import ml_dtypes
    bf = ml_dtypes.bfloat16
    const = np.zeros((128, 289), dtype=bf)
    const[:, 0:K] = W.astype(bf)                      # wblk cols 0..25
    expT = np.exp(T.astype(np.float64)).astype(bf)
    for g in range(GROUPS):
        const[32 * g:32 * g + K, 32 + 32 * g:32 + 32 * g + K] = expT  # BD
        for r in range(K):
            const[32 * g + r, 160 + 32 * g:160 + 32 * g + 32] = 1     # ZS
    kidx = np.where(np.arange(128) % 32 < K,
                    (np.arange(128) % 32).astype(np.float32), 255.0)
    const[:, 288] = kidx.astype(bf)
    return const


def _make_in_maps(X, y, W, T):
    import ml_dtypes
    bf = ml_dtypes.bfloat16
    f8 = ml_dtypes.float8_e4m3
    const = _make_consts(np.asarray(W, np.float32), np.asarray(T, np.float32))

    Xq = np.asarray(X, np.float32).astype(f8)         # [B, L, F]
    # -> [cores, F, GROUPS, L, GB] -> [cores, 128, GROUPS*L*GB]
    Xr = Xq.reshape(N_CORES, GROUPS, GB, L, F).transpose(0, 4, 1, 3, 2)
    Xr = np.ascontiguousarray(Xr).reshape(N_CORES, F, GROUPS * NT)

    yi = np.asarray(y, np.int32).reshape(N_CORES, GROUPS, GB, L)
    ya = np.ascontiguousarray(yi.transpose(0, 1, 3, 2)).reshape(
        N_CORES, GROUPS, NT).astype(bf)

    in_maps = []
    for c in range(N_CORES):
        in_maps.append({
            "XT": np.ascontiguousarray(Xr[c]),
            "YARR": np.ascontiguousarray(ya[c]),
            "CONST": const,
        })
    return in_maps


def _trans_per_core(y, T):
    y = np.asarray(y)
    T = np.asarray(T, np.float64)
    out = []
    for c in range(N_CORES):
        yc = y[c * BC:(c + 1) * BC]
        out.append(float(T[yc[:, :-1], yc[:, 1:]].sum()))
    return out


def _combine(results, trans):
    total = 0.0
    for c, r in enumerate(results):
        o = np.asarray(r["out"], dtype=np.float64)
        sumlog = o[:, 0:8].sum() / 32.0
        emit = o[:, 8:16].sum()
        total += emit + trans[c] - sumlog - BC * L * SHIFT
    return np.float32(total / B)


def kernel(X, y, W, T):
    from concourse.bass_utils import run_bass_kernel_spmd
    nc = _get_program()
    X, y = np.asarray(X), np.asarray(y)
    W, T = np.asarray(W), np.asarray(T)
    in_maps = _make_in_maps(X, y, W, T)
    res = run_bass_kernel_spmd(nc, in_maps, list(range(N_CORES)))
    return _combine(res.results, _trans_per_core(y, T))


# revision 7
# speedup vs baseline: 1.1404x; 1.1404x over previous
"""CRF loss kernel for Trainium2 (8 NeuronCores, data-parallel over batch).

Reference computation:
    score = einsum('blf,fk->blk', X, W);  forward CRF recursion over L;
    loss = mean_b(emit + trans - logZ).

Design (v2) — minimize NEFF input bytes and instruction count:
  - X is pre-transposed and quantized to fp8e4 on the host:
    XT[f, g*8192 + t*256 + b] = X[core*1024 + g*256 + b, t, f].
    One [128, 32768] fp8 DMA per core (4.2 MB vs 16.8 MB f32), no
    on-device transposes.
  - score matmul: lhsT = W block [128,32] bf16, rhs = XT 512-col chunks,
    4 label-row groups packed at partition offsets 0/32/64/96
    (tile_position).  8 PSUM rounds of [128,1024] f32.
  - expsc = exp(score - SHIFT) via one ACT per round (PSUM -> SBUF bf16).
  - emit = sum(score[gold]): per round, mask * score (DVE mult from PSUM)
    then tensor_reduce(add) into an f32 accumulator column.
    mask = is_equal(yrep, kidx) built on device from a tiny [4, 8192]
    bf16 y input (partition-broadcast DMA, stride-0 source).
  - CRF forward recursion in probability domain:
      p_t = (BD^T p_{t-1}) * expsc_t,  BD = block-diag(exp(T)) bf16,
    renormalized every RN=4 steps (z = ZS^T p group-sums, ln(z)
    accumulated via ACT Ln accum_out).
  - trans = sum T[y_t, y_{t+1}] is computed on the host (y and T only).
  - per-core output: [128, 16] f32 = 8 ln-accum cols + 8 emit-accum cols;
    host combines: loss = (sum emit + trans - sum(ln)/32 - BC*L*SHIFT)/B.
"""

import numpy as np

B, L, F, K = 8192, 32, 128, 26
N_CORES = 8
BC = B // N_CORES            # 1024 batch per core
GROUPS = 4                   # label-row groups packed on partitions
GB = BC // GROUPS            # 256 batch columns per group
NT = L * GB                  # 8192 columns in expsc/mask layout
SHIFT = 26.0
RN = 4                       # renormalize every RN recursion steps
PSC = 1024                   # PSUM round width (f32 cols, 2 banks)
NROUND = NT // PSC           # 8 score rounds
NLN = (L - 1) // RN + 1      # 7 renorms + 1 final ln column

_cache = {}


def _build_program():
    import concourse.bass as bass  # noqa: F401
    import concourse.bacc as bacc
    import concourse.tile as tile
    from concourse import mybir
    from contextlib import ExitStack

    f32 = mybir.dt.float32
    bf16 = mybir.dt.bfloat16
    fp8 = mybir.dt.float8e4
    AF = mybir.ActivationFunctionType
    ALU = mybir.AluOpType

    nc = bacc.Bacc("TRN2", target_bir_lowering=False)

    XTd = nc.dram_tensor("XT", [128, GROUPS * NT], fp8, kind="ExternalInput")
    YARRd = nc.dram_tensor("YARR", [GROUPS, NT], bf16, kind="ExternalInput")
    CONSTd = nc.dram_tensor("CONST", [128, 289], bf16, kind="ExternalInput")
    OUTd = nc.dram_tensor("out", [128, 16], f32, kind="ExternalOutput")

    with tile.TileContext(nc) as tc, ExitStack() as ctx:
        singles = ctx.enter_context(tc.tile_pool(name="singles", bufs=1))

        xt = singles.tile([128, GROUPS * NT], fp8)
        nc.sync.dma_start(out=xt, in_=XTd.ap())
        cst = singles.tile([128, 289], bf16)
        nc.sync.dma_start(out=cst, in_=CONSTd.ap())
        wblk = cst[:, 0:32]
        bd = cst[:, 32:160]
        zs = cst[:, 160:288]
        kidx = cst[:, 288:289]

        yrep = singles.tile([128, NT], bf16)
        for g in range(GROUPS):
            nc.sync.dma_start(
                out=yrep[32 * g:32 * (g + 1)],
                in_=YARRd.ap()[g:g + 1, :].to_broadcast([32, NT]),
            )
        mask = singles.tile([128, NT], bf16)
        nc.vector.tensor_tensor(
            mask, yrep, kidx.to_broadcast([128, NT]), ALU.is_equal
        )

        expsc = singles.tile([128, NT], bf16)
        acc = singles.tile([128, 16], f32)
        nshift = singles.tile([128, 1], f32)
        nc.vector.memset(nshift, -SHIFT)

        # ---------------- phase 1: score / exp / emit ----------------
        with tc.tile_pool(name="scp", bufs=2, space="PSUM") as scp, \
             tc.tile_pool(name="emp", bufs=2) as emp:
            for r in range(NROUND):
                ps = scp.tile([128, PSC], f32)
                for g in range(GROUPS):
                    for j in range(PSC // 512):
                        col = g * NT + r * PSC + j * 512
                        nc.tensor.matmul(
                            ps[32 * g:32 * g + 32, j * 512:(j + 1) * 512],
                            lhsT=wblk,
                            rhs=xt[:, col:col + 512],
                            start=True, stop=True,
                            tile_position=(0, 32 * g),
                        )
                nc.scalar.activation(
                    expsc[:, r * PSC:(r + 1) * PSC], ps, AF.Exp, bias=nshift
                )
                emsc = emp.tile([128, PSC], bf16)
                nc.vector.scalar_tensor_tensor(
                    out=emsc, in0=ps, scalar=1.0,
                    in1=mask[:, r * PSC:(r + 1) * PSC],
                    op0=ALU.mult, op1=ALU.mult,
                    accum_out=acc[:, 8 + r:9 + r],
                )

        # ---------------- phase 2: CRF recursion ----------------
        with tc.tile_pool(name="pp", bufs=2) as pp, \
             tc.tile_pool(name="vp", bufs=2) as vp, \
             tc.tile_pool(name="rzp", bufs=2) as rzp, \
             tc.tile_pool(name="lnp", bufs=2) as lnp, \
             tc.tile_pool(name="up", bufs=2, space="PSUM") as up, \
             tc.tile_pool(name="zp", bufs=2, space="PSUM") as zp:
            p_prev = expsc[:, 0:GB]
            nidx = 0
            for t in range(1, L):
                u = up.tile([128, GB], f32)
                nc.tensor.matmul(u, lhsT=bd, rhs=p_prev, start=True, stop=True)
                e_sl = expsc[:, t * GB:(t + 1) * GB]
                if t % RN == 0:
                    v = vp.tile([128, GB], bf16)
                    nc.vector.tensor_mul(v, u, e_sl)
                    z = zp.tile([128, GB], f32)
                    nc.tensor.matmul(z, lhsT=zs, rhs=v, start=True, stop=True)
                    lnscr = lnp.tile([128, GB], bf16)
                    nc.scalar.activation(
                        lnscr, z, AF.Ln, accum_out=acc[:, nidx:nidx + 1]
                    )
                    nidx += 1
                    rz = rzp.tile([128, GB], f32)
                    nc.vector.reciprocal(rz, z)
                    pn = pp.tile([128, GB], bf16)
                    nc.vector.tensor_mul(pn, v, rz)
                else:
                    pn = pp.tile([128, GB], bf16)
                    nc.vector.tensor_mul(pn, u, e_sl)
                p_prev = pn
            zf = zp.tile([128, GB], f32)
            nc.tensor.matmul(zf, lhsT=zs, rhs=p_prev, start=True, stop=True)
            lnscr = lnp.tile([128, GB], bf16)
            nc.scalar.activation(
                lnscr, zf, AF.Ln, accum_out=acc[:, nidx:nidx + 1]
            )
            nidx += 1
            assert nidx == NLN

        nc.sync.dma_start(out=OUTd.ap(), in_=acc)

    nc.compile()
    return nc


def _get_program():
    if "nc" not in _cache:
        _cache["nc"] = _build_program()
    return _cache["nc"]


def _make_consts(W, T):
    import ml_dtypes
    bf = ml_dtypes.bfloat16
    const = np.zeros((128, 289), dtype=bf)
    const[:, 0:K] = W.astype(bf)                      # wblk cols 0..25
    expT = np.exp(T.astype(np.float64)).astype(bf)
    for g in range(GROUPS):
        const[32 * g:32 * g + K, 32 + 32 * g:32 + 32 * g + K] = expT  # BD
        for r in range(K):
            const[32 * g + r, 160 + 32 * g:160 + 32 * g + 32] = 1     # ZS
    kidx = np.where(np.arange(128) % 32 < K,
                    (np.arange(128) % 32).astype(np.float32), 255.0)
    const[:, 288] = kidx.astype(bf)
    return const


def _make_in_maps(X, y, W, T):
    import ml_dtypes
    bf = ml_dtypes.bfloat16
    f8 = ml_dtypes.float8_e4m3
    const = _make_consts(np.asarray(W, np.float32), np.asarray(T, np.float32))

    Xq = np.asarray(X, np.float32).astype(f8)         # [B, L, F]
    # -> [cores, F, GROUPS, L, GB] -> [cores, 128, GROUPS*L*GB]
    Xr = Xq.reshape(N_CORES, GROUPS, GB, L, F).transpose(0, 4, 1, 3, 2)
    Xr = np.ascontiguousarray(Xr).reshape(N_CORES, F, GROUPS * NT)

    yi = np.asarray(y, np.int32).reshape(N_CORES, GROUPS, GB, L)
    ya = np.ascontiguousarray(yi.transpose(0, 1, 3, 2)).reshape(
        N_CORES, GROUPS, NT).astype(bf)

    in_maps = []
    for c in range(N_CORES):
        in_maps.append({
            "XT": np.ascontiguousarray(Xr[c]),
            "YARR": np.ascontiguousarray(ya[c]),
            "CONST": const,
        })
    return in_maps


def _trans_per_core(y, T):
    y = np.asarray(y)
    T = np.asarray(T, np.float64)
    out = []
    for c in range(N_CORES):
        yc = y[c * BC:(c + 1) * BC]
        out.append(float(T[yc[:, :-1], yc[:, 1:]].sum()))
    return out


def _combine(results, trans):
    total = 0.0
    for c, r in enumerate(results):
        o = np.asarray(r["out"], dtype=np.float64)
        sumlog = o[:, 0:8].sum() / 32.0
        emit = o[:, 8:16].sum()
        total += emit + trans[c] - sumlog - BC * L * SHIFT
    return np.float32(total / B)


def kernel(X, y, W, T):
    from concourse.bass_utils import run_bass_kernel_spmd
    nc = _get_program()
    X, y = np.asarray(X), np.asarray(y)
    W, T = np.asarray(W), np.asarray(T)
    in_maps = _make_in_maps(X, y, W, T)
    res = run_bass_kernel_spmd(nc, in_maps, list(range(N_CORES)))
    return _combine(res.results, _trans_per_core(y, T))
